# revision 25
# baseline (speedup 1.0000x reference)
"""Trainium2 Bass kernel for nn_ContactPredictionHead.

Reference computation (B=2, L=2048, D=1536, T=2):
    Wp, Wd = W[:, :D], W[:, D:]
    prod[b,i,j,t] = sum_d h[b,i,d] * Wp[t,d] * h[b,j,d]
    diff[b,i,j,t] = (h@Wd.T)[b,i,t] - (h@Wd.T)[b,j,t]
    out = symmetrize(prod + diff + bias)

Key identity: prod is symmetric in (i,j) and diff is antisymmetric, so the
symmetrization leaves   out[b,i,j,t] = prod[b,i,j,t] + bias[t]   exactly —
a weighted Gram matrix, 4 big matmuls ([2048,1536] @ [1536,2048]).

Symmetry is exploited: only the block-upper-triangle of each L x L Gram
matrix is computed on device; the host mirrors the strict lower triangle.

Sharding: 4 cores per batch item.  Core cc (0..3) of a batch receives
ht = roll(h[b].T, -128*cc, axis=1).  In this rotated frame every core runs
the SAME program: its stationary row-slots s=0..3 are local columns
[512s, 512s+128) (global rows 512s + 128cc — so slot s on the 4 cores
covers global row blocks 4s+cc, a balanced interleave of the triangle),
and it computes blocks (s, v) for v >= s against local j-chunks v.  In
global terms each row gets every j >= i covered (the v-arc from the row's
own slot wraps around), so the host can fill j < i by transposition.

The whole pipeline runs in bfloat16 (f32 PSUM accumulation): same PE rate
as float32r (1 row/cycle) but half the HBM traffic, half the LDWEIGHTS
cost and half the SBUF footprint, so the PE never starves on the input
stream.  Measured end-to-end rel err ~3.7e-3.
"""
import sys

sys.path.insert(0, "/opt/trn_rl_repo")

import numpy as np

B, L, D, T = 2, 2048, 1536, 2
NCORES = 8
CPB = NCORES // B     # cores per batch item = 4
NK = D // 128         # contraction k-tiles = 12
NJ = 512              # j columns per matmul (one PSUM bank of fp32)
NNB = L // NJ         # j chunks = 4
NS = 4                # stationary row slots per core (128 rows each)
BLOCKS = [(s, v) for v in range(NNB) for s in range(v + 1)]   # 10 blocks
# DMA part boundaries (k-tile indices) per chunk: chunk 0 leads with a
# small k0:2 slice so the first a-scale + matmul can fire early.
PARTS = {0: [0, 2, 5, 8, NK]}
for _v in range(1, NNB):
    PARTS[_v] = [0, 6, NK]

_CACHE = {}


def _get_nc():
    if "nc" in _CACHE:
        return _CACHE["nc"]
    import concourse.tile as tile
    from concourse.tile_rust import add_dep_helper
    from concourse import bacc, mybir

    f32, bf16 = mybir.dt.float32, mybir.dt.bfloat16
    nc = bacc.Bacc("TRN2", target_bir_lowering=False, debug=False,
                   num_devices=NCORES, enable_partition_id=False,
                   enable_asserts=False)
    # ht arrives pre-tiled from the host in the exact SBUF layout
    # ht[v, p, k*NJ+j] = ht_local[128k+p, 512v+j], so every chunk DMA is a
    # contiguous identity copy (12 KB runs per partition instead of 1 KB
    # rows — fewer descriptors, higher effective ring bandwidth).
    ht_d = nc.dram_tensor("ht", [NNB, 128, NK * NJ], bf16,
                          kind="ExternalInput")
    wp_d = nc.dram_tensor("wp", [128, T * NK], bf16, kind="ExternalInput")
    # Both t-planes of a block pack into one [128, 2*NJ] tile so each block
    # is a single out-DMA with 2 KB contiguous runs per partition.
    out_d = nc.dram_tensor("out", [len(BLOCKS), 128, T * NJ], bf16,
                           kind="ExternalOutput")

    with tile.TileContext(nc) as tc:
        with tc.tile_pool(name="big", bufs=1) as big, \
             tc.tile_pool(name="st", bufs=8) as stp, \
             tc.tile_pool(name="wps", bufs=1, space="PSUM") as wpsp, \
             tc.tile_pool(name="ps9", bufs=2, space="PSUM") as psp9, \
             tc.tile_pool(name="ps", bufs=5, space="PSUM") as psp:
            # Allocation order fixes SBUF addresses: stationary operands low,
            # moving operands above them (measured faster LDWEIGHTS).
            wt = big.tile([128, T * NK], bf16, name="wt")
            # a[t][p, s, k, x] = ht_local[128k+p, 512s+x] * Wp[t, 128k+p]
            a = [big.tile([128, NS, NK, 128], bf16, name=f"a{t}")
                 for t in range(T)]
            # htn[v][p, k*NJ + j] = ht_local[128k+p, 512v + j]
            htn = [big.tile([128, NK * NJ], bf16, name=f"htn{v}")
                   for v in range(NNB)]

            # wt arrives pre-gathered from the host: wp[p, t*NK+k]
            # = Wp[t, 128k+p], so this DMA is a contiguous identity copy.
            nc.scalar.dma_start(wt[:], wp_d.ap())
            # Warm the PE clock (HAM runs it at ~half speed for ~6 us from
            # the first activity) with throwaway matmuls on a scratch tile
            # memset by gpsimd, which is idle right after the framework
            # preamble — warmup starts ~0.6 us earlier than via vector.
            wdum = big.tile([128, 128], bf16, name="wdum")
            nc.gpsimd.memset(wdum[:], 0.0)
            wacc = wpsp.tile([128, 128], f32, name="wacc")
            for _ in range(38):
                nc.tensor.matmul(wacc[:], wdum[:], wdum[:], start=True,
                                 stop=True)
            # ht chunks land in fixed order on the sync (HWDGE) ring so the
            # PE can start on chunk 0 while the rest stream in.
            prev = None
            for v in range(NNB):
                bounds = PARTS[v]
                for h in range(len(bounds) - 1):
                    k0, k1 = bounds[h], bounds[h + 1]
                    dma = nc.sync.dma_start(
                        htn[v][:, k0 * NJ:k1 * NJ],
                        ht_d.ap()[v, :, k0 * NJ:k1 * NJ])
                    if prev is not None:
                        add_dep_helper(dma.ins, prev.ins, sync=False,
                                       reason="ht chunks stream in j order")
                    prev = dma

            # Stationary operands: muls split to match the chunk-s DMA parts,
            # reading columns 0:128 of every k-block of htn[s]; the per-(p,k)
            # scale comes from a stride-0 broadcast of wt along x.
            for s in range(NS):
                src = htn[s][:].rearrange("p (k j) -> p k j", k=NK)
                bounds = PARTS[s]
                for h in range(len(bounds) - 1):
                    ks = slice(bounds[h], bounds[h + 1])
                    kq = bounds[h + 1] - bounds[h]
                    for t in range(T):
                        scale = (wt[:, t * NK + bounds[h]: t * NK + bounds[h + 1]]
                                 .unsqueeze(2).broadcast_to([128, kq, 128]))
                        nc.vector.tensor_mul(
                            a[t][:, s, ks], src[:, ks, 0:128], scale)

            for bi, (s, v) in enumerate(BLOCKS):
                st = stp.tile([128, T * NJ], bf16, name="st", tag="st")
                last = bi == len(BLOCKS) - 1
                for t in range(T):
                    if last and t == 1:
                        continue
                    acc = psp.tile([128, NJ], f32, name="acc", tag="acc")
                    for k in range(NK):
                        nc.tensor.matmul(
                            acc[:], a[t][:, s, k], htn[v][:, k * NJ:(k + 1) * NJ],
                            start=(k == 0), stop=(k == NK - 1))
                    if t == 0:
                        nc.vector.tensor_copy(st[:, 0:NJ], acc[:])
                    else:
                        nc.scalar.copy(st[:, NJ:T * NJ], acc[:])
                if not last:
                    # Outs split across both DMA rings: early blocks on the
                    # (slow) gpsimd ring where deadlines are loose, late
                    # blocks on the scalar HWDGE ring (input-free by then)
                    # so the final drain is short.
                    out_eng = nc.scalar if bi >= 6 else nc.gpsimd
                    out_eng.dma_start(out_d.ap()[bi], st[:])
                    continue
                # Final block: t0 ships as soon as its copy lands, and the
                # very last group (t=1) runs as two 256-column half-groups
                # in separate PSUM banks so the first half's copy + DMA
                # drain while the PE computes the second half — the only
                # fully-exposed drain left is ~131 KB.
                nc.scalar.dma_start(out_d.ap()[bi, :, 0:NJ], st[:, 0:NJ])
                for half in range(2):
                    o = NJ + 256 * half
                    acc = psp9.tile([128, 256], f32, name="acc9", tag="acc9")
                    for k in range(NK):
                        nc.tensor.matmul(
                            acc[:], a[1][:, s, k],
                            htn[v][:, k * NJ + 256 * half:k * NJ + 256 * (half + 1)],
                            start=(k == 0), stop=(k == NK - 1))
                    if half == 0:
                        nc.vector.tensor_copy(st[:, o:o + 256], acc[:])
                    else:
                        nc.scalar.copy(st[:, o:o + 256], acc[:])
                    nc.scalar.dma_start(out_d.ap()[bi, :, o:o + 256],
                                        st[:, o:o + 256])
    nc.compile()
    _CACHE["nc"] = nc
    return nc


def make_in_maps(h, W):
    import ml_dtypes
    bf = ml_dtypes.bfloat16
    # wp[p, t*NK+k] = Wp[t, 128k+p]
    wp = np.ascontiguousarray(
        W[:, :D].reshape(T, NK, 128).transpose(2, 0, 1).reshape(128, T * NK)
    ).astype(bf)
    hts = [np.ascontiguousarray(h[bi].T).astype(bf) for bi in range(B)]  # [D, L]
    in_maps = []
    for c in range(NCORES):
        bi, r = c // CPB, (c % CPB) * 128
        ht = hts[bi] if r == 0 else np.roll(hts[bi], -r, axis=1)
        # ht2[v, p, k*NJ+j] = ht[128k+p, 512v+j]
        ht2 = np.ascontiguousarray(
            np.asarray(ht).reshape(NK, 128, NNB, NJ).transpose(2, 1, 0, 3)
        ).reshape(NNB, 128, NK * NJ)
        in_maps.append({"ht": ht2, "wp": wp})
    return in_maps


def kernel(hidden_states, W, b):
    from concourse.bass_utils import run_bass_kernel_spmd

    h = np.ascontiguousarray(hidden_states, dtype=np.float32)
    W = np.asarray(W, dtype=np.float32)
    bias = np.asarray(b, dtype=np.float32)
    nc = _get_nc()

    res = run_bass_kernel_spmd(nc, make_in_maps(h, W),
                               core_ids=list(range(NCORES)))
    full = np.empty((B, L, L, T), np.float32)
    for c in range(NCORES):
        bi, r = c // CPB, (c % CPB) * 128
        # [len(BLOCKS), 128, T*NJ] -> [len(BLOCKS), 128, NJ, T]
        blocks = (res.results[c]["out"].astype(np.float32)
                  .reshape(len(BLOCKS), 128, T, NJ).transpose(0, 1, 3, 2))
        for idx, (s, v) in enumerate(BLOCKS):
            rows = slice(512 * s + r, 512 * s + r + 128)
            g = (512 * v + r) % L
            blk = blocks[idx]
            if g + NJ <= L:
                full[bi, rows, g:g + NJ] = blk
            else:
                w = L - g
                full[bi, rows, g:] = blk[:, :w]
                full[bi, rows, :NJ - w] = blk[:, w:]
    # Mirror: keep computed j >= i, take j < i from the transpose.
    idx = np.arange(L)
    mask = (idx[None, :] >= idx[:, None])[None, :, :, None]
    out = np.where(mask, full, full.transpose(0, 2, 1, 3))
    if np.any(bias != 0):
        out += bias
    return out


# revision 26
# speedup vs baseline: 1.0013x; 1.0013x over previous
"""Trainium2 Bass kernel for nn_ContactPredictionHead.

Reference computation (B=2, L=2048, D=1536, T=2):
    Wp, Wd = W[:, :D], W[:, D:]
    prod[b,i,j,t] = sum_d h[b,i,d] * Wp[t,d] * h[b,j,d]
    diff[b,i,j,t] = (h@Wd.T)[b,i,t] - (h@Wd.T)[b,j,t]
    out = symmetrize(prod + diff + bias)

Key identity: prod is symmetric in (i,j) and diff is antisymmetric, so the
symmetrization leaves   out[b,i,j,t] = prod[b,i,j,t] + bias[t]   exactly —
a weighted Gram matrix, 4 big matmuls ([2048,1536] @ [1536,2048]).

Symmetry is exploited: only the block-upper-triangle of each L x L Gram
matrix is computed on device; the host mirrors the strict lower triangle.

Sharding: 4 cores per batch item.  Core cc (0..3) of a batch receives
ht = roll(h[b].T, -128*cc, axis=1).  In this rotated frame every core runs
the SAME program: its stationary row-slots s=0..3 are local columns
[512s, 512s+128) (global rows 512s + 128cc — so slot s on the 4 cores
covers global row blocks 4s+cc, a balanced interleave of the triangle),
and it computes blocks (s, v) for v >= s against local j-chunks v.  In
global terms each row gets every j >= i covered (the v-arc from the row's
own slot wraps around), so the host can fill j < i by transposition.

The whole pipeline runs in bfloat16 (f32 PSUM accumulation): same PE rate
as float32r (1 row/cycle) but half the HBM traffic, half the LDWEIGHTS
cost and half the SBUF footprint, so the PE never starves on the input
stream.  Measured end-to-end rel err ~3.7e-3.
"""
import sys

sys.path.insert(0, "/opt/trn_rl_repo")

import numpy as np

B, L, D, T = 2, 2048, 1536, 2
NCORES = 8
CPB = NCORES // B     # cores per batch item = 4
NK = D // 128         # contraction k-tiles = 12
NJ = 512              # j columns per matmul (one PSUM bank of fp32)
NNB = L // NJ         # j chunks = 4
NS = 4                # stationary row slots per core (128 rows each)
BLOCKS = [(s, v) for v in range(NNB) for s in range(v + 1)]   # 10 blocks
# DMA part boundaries (k-tile indices) per chunk: chunk 0 leads with a
# small k0:2 slice so the first a-scale + matmul can fire early.
PARTS = {0: [0, 2, 5, 8, NK]}
for _v in range(1, NNB):
    PARTS[_v] = [0, 6, NK]

_CACHE = {}


def _get_nc():
    if "nc" in _CACHE:
        return _CACHE["nc"]
    import concourse.tile as tile
    from concourse.tile_rust import add_dep_helper
    from concourse import bacc, mybir

    f32, bf16 = mybir.dt.float32, mybir.dt.bfloat16
    nc = bacc.Bacc("TRN2", target_bir_lowering=False, debug=False,
                   num_devices=NCORES, enable_partition_id=False,
                   enable_asserts=False)
    # ht arrives pre-tiled from the host in the exact SBUF layout
    # ht[v, p, k*NJ+j] = ht_local[128k+p, 512v+j], so every chunk DMA is a
    # contiguous identity copy (12 KB runs per partition instead of 1 KB
    # rows — fewer descriptors, higher effective ring bandwidth).
    ht_d = nc.dram_tensor("ht", [NNB, 128, NK * NJ], bf16,
                          kind="ExternalInput")
    wp_d = nc.dram_tensor("wp", [128, T * NK], bf16, kind="ExternalInput")
    # Both t-planes of a block pack into one [128, 2*NJ] tile so each block
    # is a single out-DMA with 2 KB contiguous runs per partition.
    out_d = nc.dram_tensor("out", [len(BLOCKS), 128, T * NJ], bf16,
                           kind="ExternalOutput")

    with tile.TileContext(nc) as tc:
        with tc.tile_pool(name="big", bufs=1) as big, \
             tc.tile_pool(name="st", bufs=8) as stp, \
             tc.tile_pool(name="wps", bufs=1, space="PSUM") as wpsp, \
             tc.tile_pool(name="ps9", bufs=2, space="PSUM") as psp9, \
             tc.tile_pool(name="ps", bufs=5, space="PSUM") as psp:
            # Allocation order fixes SBUF addresses: stationary operands low,
            # moving operands above them (measured faster LDWEIGHTS).
            wt = big.tile([128, T * NK], bf16, name="wt")
            # a[t][p, s, k, x] = ht_local[128k+p, 512s+x] * Wp[t, 128k+p]
            a = [big.tile([128, NS, NK, 128], bf16, name=f"a{t}")
                 for t in range(T)]
            # htn[v][p, k*NJ + j] = ht_local[128k+p, 512v + j]
            htn = [big.tile([128, NK * NJ], bf16, name=f"htn{v}")
                   for v in range(NNB)]

            # wt arrives pre-gathered from the host: wp[p, t*NK+k]
            # = Wp[t, 128k+p], so this DMA is a contiguous identity copy.
            nc.scalar.dma_start(wt[:], wp_d.ap())
            # Warm the PE clock (HAM runs it at ~half speed for ~6 us from
            # the first activity) with throwaway matmuls on a scratch tile
            # memset by gpsimd, which is idle right after the framework
            # preamble — warmup starts ~0.6 us earlier than via vector.
            wdum = big.tile([128, 128], bf16, name="wdum")
            nc.gpsimd.memset(wdum[:], 0.0)
            wacc = wpsp.tile([128, 128], f32, name="wacc")
            for _ in range(38):
                nc.tensor.matmul(wacc[:], wdum[:], wdum[:], start=True,
                                 stop=True)
            # ht chunks land in fixed order on the sync (HWDGE) ring so the
            # PE can start on chunk 0 while the rest stream in.
            prev = None
            for v in range(NNB):
                bounds = PARTS[v]
                for h in range(len(bounds) - 1):
                    k0, k1 = bounds[h], bounds[h + 1]
                    dma = nc.sync.dma_start(
                        htn[v][:, k0 * NJ:k1 * NJ],
                        ht_d.ap()[v, :, k0 * NJ:k1 * NJ])
                    if prev is not None:
                        add_dep_helper(dma.ins, prev.ins, sync=False,
                                       reason="ht chunks stream in j order")
                    prev = dma

            # Stationary operands: muls split to match the chunk-s DMA parts,
            # reading columns 0:128 of every k-block of htn[s]; the per-(p,k)
            # scale comes from a stride-0 broadcast of wt along x.
            # t0 muls for a slot all come before its t1 muls: the t0 k-chain
            # is the critical path to the first matmuls of each block, while
            # t1 stationaries are not needed until ~2.6 us later.
            for s in range(NS):
                src = htn[s][:].rearrange("p (k j) -> p k j", k=NK)
                bounds = PARTS[s]
                for t in range(T):
                    for h in range(len(bounds) - 1):
                        ks = slice(bounds[h], bounds[h + 1])
                        kq = bounds[h + 1] - bounds[h]
                        scale = (wt[:, t * NK + bounds[h]: t * NK + bounds[h + 1]]
                                 .unsqueeze(2).broadcast_to([128, kq, 128]))
                        nc.vector.tensor_mul(
                            a[t][:, s, ks], src[:, ks, 0:128], scale)

            for bi, (s, v) in enumerate(BLOCKS):
                st = stp.tile([128, T * NJ], bf16, name="st", tag="st")
                last = bi == len(BLOCKS) - 1
                for t in range(T):
                    if last and t == 1:
                        continue
                    acc = psp.tile([128, NJ], f32, name="acc", tag="acc")
                    for k in range(NK):
                        nc.tensor.matmul(
                            acc[:], a[t][:, s, k], htn[v][:, k * NJ:(k + 1) * NJ],
                            start=(k == 0), stop=(k == NK - 1))
                    if t == 0:
                        nc.vector.tensor_copy(st[:, 0:NJ], acc[:])
                    else:
                        nc.scalar.copy(st[:, NJ:T * NJ], acc[:])
                if not last:
                    # Outs split across both DMA rings: early blocks on the
                    # (slow) gpsimd ring where deadlines are loose, late
                    # blocks on the scalar HWDGE ring (input-free by then)
                    # so the final drain is short.
                    out_eng = nc.scalar if bi >= 6 else nc.gpsimd
                    out_eng.dma_start(out_d.ap()[bi], st[:])
                    continue
                # Final block: t0 ships as soon as its copy lands, and the
                # very last group (t=1) runs as two 256-column half-groups
                # in separate PSUM banks so the first half's copy + DMA
                # drain while the PE computes the second half — the only
                # fully-exposed drain left is ~131 KB.
                nc.scalar.dma_start(out_d.ap()[bi, :, 0:NJ], st[:, 0:NJ])
                for half in range(2):
                    o = NJ + 256 * half
                    acc = psp9.tile([128, 256], f32, name="acc9", tag="acc9")
                    for k in range(NK):
                        nc.tensor.matmul(
                            acc[:], a[1][:, s, k],
                            htn[v][:, k * NJ + 256 * half:k * NJ + 256 * (half + 1)],
                            start=(k == 0), stop=(k == NK - 1))
                    if half == 0:
                        nc.vector.tensor_copy(st[:, o:o + 256], acc[:])
                    else:
                        nc.scalar.copy(st[:, o:o + 256], acc[:])
                    nc.scalar.dma_start(out_d.ap()[bi, :, o:o + 256],
                                        st[:, o:o + 256])
    nc.compile()
    _CACHE["nc"] = nc
    return nc


def make_in_maps(h, W):
    import ml_dtypes
    bf = ml_dtypes.bfloat16
    # wp[p, t*NK+k] = Wp[t, 128k+p]
    wp = np.ascontiguousarray(
        W[:, :D].reshape(T, NK, 128).transpose(2, 0, 1).reshape(128, T * NK)
    ).astype(bf)
    hts = [np.ascontiguousarray(h[bi].T).astype(bf) for bi in range(B)]  # [D, L]
    in_maps = []
    for c in range(NCORES):
        bi, r = c // CPB, (c % CPB) * 128
        ht = hts[bi] if r == 0 else np.roll(hts[bi], -r, axis=1)
        # ht2[v, p, k*NJ+j] = ht[128k+p, 512v+j]
        ht2 = np.ascontiguousarray(
            np.asarray(ht).reshape(NK, 128, NNB, NJ).transpose(2, 1, 0, 3)
        ).reshape(NNB, 128, NK * NJ)
        in_maps.append({"ht": ht2, "wp": wp})
    return in_maps


def kernel(hidden_states, W, b):
    from concourse.bass_utils import run_bass_kernel_spmd

    h = np.ascontiguousarray(hidden_states, dtype=np.float32)
    W = np.asarray(W, dtype=np.float32)
    bias = np.asarray(b, dtype=np.float32)
    nc = _get_nc()

    res = run_bass_kernel_spmd(nc, make_in_maps(h, W),
                               core_ids=list(range(NCORES)))
    full = np.empty((B, L, L, T), np.float32)
    for c in range(NCORES):
        bi, r = c // CPB, (c % CPB) * 128
        # [len(BLOCKS), 128, T*NJ] -> [len(BLOCKS), 128, NJ, T]
        blocks = (res.results[c]["out"].astype(np.float32)
                  .reshape(len(BLOCKS), 128, T, NJ).transpose(0, 1, 3, 2))
        for idx, (s, v) in enumerate(BLOCKS):
            rows = slice(512 * s + r, 512 * s + r + 128)
            g = (512 * v + r) % L
            blk = blocks[idx]
            if g + NJ <= L:
                full[bi, rows, g:g + NJ] = blk
            else:
                w = L - g
                full[bi, rows, g:] = blk[:, :w]
                full[bi, rows, :NJ - w] = blk[:, w:]
    # Mirror: keep computed j >= i, take j < i from the transpose.
    idx = np.arange(L)
    mask = (idx[None, :] >= idx[:, None])[None, :, :, None]
    out = np.where(mask, full, full.transpose(0, 2, 1, 3))
    if np.any(bias != 0):
        out += bias
    return out
